# revision 17
# baseline (speedup 1.0000x reference)
"""Trainium2 Bass kernel for nn_CGWeight (CG-weighted bilinear message passing).

out[e, k] = sum_c w_c * einsum('ijk,ei,ej->ek', cg_c, a_{l1(c)}, h_{l2(c)})

Strategy (data-parallel over E across 8 cores; per core):
  Fold weight+cg into B[i,j,k] (9x9x3) on host; out[e,k] = x_e^T B_k y_e with
  x = concat(a0,a1,a2), y = concat(h0,h1,h2) (9 features each).
  Host pre-transposes x,y to a slot-interleaved feature-major layout
  [126, NBLK] (14 edge "slots" x 9 features on partitions), so the device
  needs no on-chip transposes and the PE streams 14 edges per cycle-column.
  Per 512-column tile, for each output component k:
    G_k  = CGK_k.T @ y_tile     (PE, fp16)      G_k[(s,i),n] = sum_j B[i,j,k] y_j
    prod = G_k * x_tile         (DVE k=0,1; ACT-copy + GPSIMD k=2)
    out += SUMR_k.T @ prod      (PE, narrow 42-col stationary; the two tiles
                                 of a pair land at PSUM partition 0 / 64 via
                                 quadrant tile positions)
  SUM matmuls are emitted one tile behind the G/product stream so the PE
  queue never head-of-line blocks on unfinished products (lets LDWEIGHTS
  pull ahead and hide). Output stored fp16, assembled [106, 1024] and DMA'd
  per 4 tiles. Input chunks ramp [1,1,2,4,8,...] tiles so compute starts as
  soon as the first 258 KB lands.
"""
import numpy as np

import concourse.bass as bass
import concourse.mybir as mybir
from concourse import tile
from concourse.bass_utils import run_bass_kernel_spmd

E = 3_200_000
N_CORES = 8
E_CORE = E // N_CORES          # 400_000
S = 14                         # edge slots per matmul column
P = 9 * S                      # 126 partitions
TILE_N = 512                   # matmul free dim
T = 56                         # tiles per core: 14*56*512 = 401408 >= 400000
NBLK = T * TILE_N              # 28672 edges per slot
E_PAD = S * NBLK               # 401408
CHUNKS = [1, 1, 2, 2] + [4] * 12 + [2]   # tiles per input DMA (ramped prefetch)
assert sum(CHUNKS) == T
TGRP = 2                       # tiles per opsum group ([84, 512] PSUM)
N_GRP = T // TGRP              # 28
OROW = 84                      # opsum rows: tm*42 + s*3 + k
OGRP = 2                       # opsum groups per output DMA ([84, 1024] fp16)

COMBOS = [(0, 1), (1, 0), (1, 1), (1, 2), (2, 1), (2, 2)]
OFF = {0: 0, 1: 1, 2: 4}
DIM = {0: 1, 1: 3, 2: 5}

_F32 = mybir.dt.float32
_F32R = mybir.dt.float32r
_F16 = mybir.dt.float16
_YDT = _F16
_XDT = _F16
_ODT = _F16


def _split_multi_waits(nc, max_waits=1):
    """walrus CoreV3 setupSyncWait only accepts one sync-wait per
    instruction; hoist extra waits onto same-engine NoOps placed before."""
    ctr = 0
    for fn in nc.m.functions:
        for blk in fn.blocks:
            out = []
            changed = False
            for ins in blk.instructions:
                si = getattr(ins, "sync_info", None)
                waits = list(si.on_wait) if si is not None else []
                if len(waits) > max_waits:
                    changed = True
                    keep = waits[-max_waits:]
                    for w in waits[:-max_waits]:
                        ctr += 1
                        out.append(mybir.InstNoOp(
                            name=f"I-waitsplit-{ctr}",
                            engine=ins.engine,
                            ins=[], outs=[],
                            sync_info=mybir.SyncInfo(on_wait=[w], on_update=[]),
                        ))
                    ins.sync_info = mybir.SyncInfo(
                        on_wait=keep, on_update=list(si.on_update))
                out.append(ins)
            if changed:
                blk.instructions = out
    return nc


def _build_nc():
    nc = bass.Bass()
    # host supplies slot-interleaved layout: row s*9+i = feature i of slot s
    xt_d = nc.dram_tensor("xt", [P, NBLK], _XDT, kind="ExternalInput")
    yt_d = nc.dram_tensor("yt", [P, NBLK], _YDT, kind="ExternalInput")
    cg_d = nc.dram_tensor("cgk", [P, 3 * P], _YDT, kind="ExternalInput")
    sumr_d = nc.dram_tensor("sumr", [P, TGRP * 3 * 84], _F32R, kind="ExternalInput")
    o_d = nc.dram_tensor("ov2", [OROW, N_GRP * TILE_N], _ODT, kind="ExternalOutput")

    # per-tile chunk-local column offsets and chunk boundaries
    tile_chunk = []      # (chunk_idx, local_col)
    for ci, csz in enumerate(CHUNKS):
        for j in range(csz):
            tile_chunk.append((ci, j * TILE_N))

    with tile.TileContext(nc) as tc:
        with (
            tc.tile_pool(name="consts", bufs=1) as cpool,
            tc.tile_pool(name="inx", bufs=3) as xpool,
            tc.tile_pool(name="iny", bufs=3) as ypool,
            tc.tile_pool(name="work", bufs=9) as wpool,
            tc.tile_pool(name="gstage", bufs=3) as spool,
            tc.tile_pool(name="outs", bufs=2) as opool,
            tc.tile_pool(name="psG", bufs=6, space="PSUM") as psG,
            tc.tile_pool(name="psO", bufs=2, space="PSUM") as psO,
        ):
            cg_sb = cpool.tile([P, 3 * P], _YDT, tag="cg")
            sumr_sb = cpool.tile([P, TGRP * 3 * 84], _F32R, tag="sumr")
            nc.scalar.dma_start(cg_sb[:], cg_d[:])

            # PE warmup: junk matmuls reading cg (lands ~9.5us) bridge the
            # gap until real tiles stream, so the HAM clock gate promotes to
            # 2.4 GHz exactly once with no demotion dip.
            for _ in range(12):
                warm = psG.tile([P, 3 * P], _F32, tag="g")
                nc.tensor.matmul(warm[:], cg_sb[:, :P], cg_sb[:],
                                 start=True, stop=True)

            xcs, ycs = {}, {}
            t_base = 0
            for ci, csz in enumerate(CHUNKS):
                xcs[ci] = (t_base, None)
                t_base += csz

            pend = None          # deferred SUM work: (tile_idx, prods, xslice)
            out_sb = None
            opsum = None
            cur_chunk = -1

            def emit_sums(t, prods):
                nonlocal out_sb, opsum
                g, tm = t // TGRP, t % TGRP
                if tm == 0:
                    opsum = psO.tile([OROW, TILE_N], _F32, tag="o")
                    if g % OGRP == 0:
                        out_sb = opool.tile(
                            [OROW, OGRP * TILE_N], _ODT, tag="osb")
                for k in range(3):
                    v = tm * 3 + k
                    nc.tensor.matmul(
                        opsum[:], sumr_sb[:, v * 84:(v + 1) * 84],
                        prods[k][:],
                        start=(v == 0), stop=(v == TGRP * 3 - 1))
                if tm == TGRP - 1:
                    og = g % OGRP
                    nc.scalar.copy(
                        out_sb[:, og * TILE_N:(og + 1) * TILE_N], opsum[:])
                    if og == OGRP - 1:
                        p = g // OGRP
                        nc.sync.dma_start(
                            o_d[:, p * OGRP * TILE_N:(p + 1) * OGRP * TILE_N],
                            out_sb[:])

            for t in range(T):
                ci, co = tile_chunk[t]
                if ci != cur_chunk:
                    cur_chunk = ci
                    csz = CHUNKS[ci]
                    c0 = xcs[ci][0] * TILE_N
                    c1 = c0 + csz * TILE_N
                    xc = xpool.tile([P, 4 * TILE_N], _XDT, tag="xc")
                    yc = ypool.tile([P, 4 * TILE_N], _YDT, tag="yc")
                    nc.scalar.dma_start(xc[:, :c1 - c0], xt_d[:, c0:c1])
                    nc.sync.dma_start(yc[:, :c1 - c0], yt_d[:, c0:c1])
                    if ci == 0:
                        # sumr is first needed by tile 0's SUM matmuls, well
                        # after the first chunk; don't delay chunk 0's issue
                        nc.sync.dma_start(sumr_sb[:], sumr_d[:])
                # G + products for tile t
                prods = []
                for k in range(3):
                    gp = psG.tile([P, TILE_N], _F32, tag="g")
                    nc.tensor.matmul(
                        gp[:], cg_sb[:, k * P:(k + 1) * P],
                        yc[:, co:co + TILE_N], start=True, stop=True)
                    prod = wpool.tile([P, TILE_N], _F32R, tag="prod")
                    if k == 2 and t < T - 2:
                        # GPSIMD can't read PSUM: ACT stages G to SBUF.
                        # Last tiles go to DVE so the tail drains fast.
                        g_sb = spool.tile([P, TILE_N], _F32, tag="gsb")
                        nc.scalar.copy(g_sb[:], gp[:])
                        nc.gpsimd.tensor_mul(
                            prod[:], g_sb[:], xc[:, co:co + TILE_N])
                    else:
                        nc.vector.tensor_mul(
                            prod[:], gp[:], xc[:, co:co + TILE_N])
                    prods.append(prod)
                # SUMs for the previous tile (1-tile software pipeline)
                if pend is not None:
                    emit_sums(*pend)
                pend = (t, prods)
            emit_sums(*pend)

    _split_multi_waits(nc)
    return nc


_NC_CACHE = None


def _get_nc():
    global _NC_CACHE
    if _NC_CACHE is None:
        _NC_CACHE = _build_nc()
    return _NC_CACHE


def _make_consts(weight, cgd):
    B = np.zeros((9, 9, 3), np.float32)
    for ci, (l1, l2) in enumerate(COMBOS):
        B[OFF[l1]:OFF[l1] + DIM[l1], OFF[l2]:OFF[l2] + DIM[l2], :] += (
            weight[ci] * cgd[(l1, l2)])
    # CGK[k][(s,j), (s,i)] = B[i,j,k]
    CGK = np.zeros((3, P, P), np.float32)
    for k in range(3):
        for s in range(S):
            CGK[k, s * 9:(s + 1) * 9, s * 9:(s + 1) * 9] = B[:, :, k].T
    cgk = np.concatenate([CGK[0], CGK[1], CGK[2]], axis=1)  # [126, 378]
    # SUMR variant (tm, k): [(s,i), tm*42 + s*3 + k] = 1
    sumr = np.zeros((P, TGRP * 3 * 84), np.float32)
    for tm in range(TGRP):
        for k in range(3):
            blk = (tm * 3 + k) * 84
            for s in range(S):
                sumr[s * 9:(s + 1) * 9, blk + tm * 42 + s * 3 + k] = 1.0
    return cgk, sumr


def _feature_major(a0, a1, a2, lo, hi):
    """[126, NBLK] slot-interleaved feature-major slice of
    concat(a0,a1,a2) rows lo:hi. Row s*9+i, col n = feature i of edge
    s*NBLK + n (relative to lo)."""
    f = np.zeros((9, E_PAD), np.float32)
    f[0, :hi - lo] = a0[lo:hi, 0]
    f[1:4, :hi - lo] = a1[lo:hi].T
    f[4:9, :hi - lo] = a2[lo:hi].T
    return np.ascontiguousarray(
        f.reshape(9, S, NBLK).transpose(1, 0, 2).reshape(P, NBLK))


def _run_spmd(inputs, trace=False):
    a0 = np.asarray(inputs["a0"], np.float32)
    a1 = np.asarray(inputs["a1"], np.float32)
    a2 = np.asarray(inputs["a2"], np.float32)
    h0 = np.asarray(inputs["h0"], np.float32)
    h1 = np.asarray(inputs["h1"], np.float32)
    h2 = np.asarray(inputs["h2"], np.float32)
    weight = np.asarray(inputs["weight"], np.float32)
    cgd = {(l1, l2): np.asarray(inputs[f"cg{l1}{l2}"], np.float32)
           for (l1, l2) in COMBOS}

    cgk, sumr = _make_consts(weight, cgd)
    ydt = mybir.dt.np(_YDT)
    xdt = mybir.dt.np(_XDT)
    cgk = cgk.astype(ydt)
    in_maps = []
    for c in range(N_CORES):
        lo, hi = c * E_CORE, (c + 1) * E_CORE
        in_maps.append({
            "xt": _feature_major(a0, a1, a2, lo, hi).astype(xdt),
            "yt": _feature_major(h0, h1, h2, lo, hi).astype(ydt),
            "cgk": cgk, "sumr": sumr,
        })

    nc = _get_nc()
    br = run_bass_kernel_spmd(nc, in_maps, list(range(N_CORES)), trace=trace)

    out = np.empty((E, 3), np.float32)
    for c in range(N_CORES):
        O = np.asarray(br.results[c]["ov2"], np.float32)  # [84, 28*512] fp16
        # row r = tm*42 + s*3 + k; col = g*512 + n; tile t = g*2 + tm
        # edge e = s*NBLK + t*512 + n
        dec = (O.reshape(TGRP, S, 3, N_GRP, TILE_N)
               .transpose(1, 3, 0, 4, 2)                # [s, g, tm, n, k]
               .reshape(E_PAD, 3))
        out[c * E_CORE:(c + 1) * E_CORE] = dec[:E_CORE]
    return out, br


def kernel(**inputs):
    out, _ = _run_spmd(inputs, trace=False)
    return out


# revision 18
# speedup vs baseline: 1.0781x; 1.0781x over previous
"""Trainium2 Bass kernel for nn_CGWeight (CG-weighted bilinear message passing).

out[e, k] = sum_c w_c * einsum('ijk,ei,ej->ek', cg_c, a_{l1(c)}, h_{l2(c)})

Strategy (data-parallel over E across 8 cores; per core):
  Fold weight+cg into B[i,j,k] (9x9x3) on host; out[e,k] = x_e^T B_k y_e with
  x = concat(a0,a1,a2), y = concat(h0,h1,h2) (9 features each).
  Host pre-transposes x,y to a slot-interleaved feature-major layout
  [126, NBLK] (14 edge "slots" x 9 features on partitions), so the device
  needs no on-chip transposes and the PE streams 14 edges per cycle-column.
  Per 512-column tile, for each output component k:
    G_k  = CGK_k.T @ y_tile     (PE, fp16)      G_k[(s,i),n] = sum_j B[i,j,k] y_j
    prod = G_k * x_tile         (DVE k=0,1; ACT-copy + GPSIMD k=2)
    out += SUMR_k.T @ prod      (PE, narrow 42-col stationary; the two tiles
                                 of a pair land at PSUM partition 0 / 64 via
                                 quadrant tile positions)
  SUM matmuls are emitted one tile behind the G/product stream so the PE
  queue never head-of-line blocks on unfinished products (lets LDWEIGHTS
  pull ahead and hide). Output stored fp16, assembled [106, 1024] and DMA'd
  per 4 tiles. Input chunks ramp [1,1,2,4,8,...] tiles so compute starts as
  soon as the first 258 KB lands.
"""
import numpy as np

import concourse.bass as bass
import concourse.mybir as mybir
from concourse import tile
from concourse.bass_utils import run_bass_kernel_spmd

E = 3_200_000
N_CORES = 8
E_CORE = E // N_CORES          # 400_000
S = 14                         # edge slots per matmul column
P = 9 * S                      # 126 partitions
TILE_N = 512                   # matmul free dim
T = 56                         # tiles per core: 14*56*512 = 401408 >= 400000
NBLK = T * TILE_N              # 28672 edges per slot
E_PAD = S * NBLK               # 401408
CHUNKS = [1, 1, 2, 2] + [4] * 12 + [2]   # tiles per input DMA (ramped prefetch)
assert sum(CHUNKS) == T
TGRP = 2                       # tiles per opsum group ([84, 512] PSUM)
N_GRP = T // TGRP              # 28
OROW = 84                      # opsum rows: tm*42 + s*3 + k
OGRP = 2                       # opsum groups per output DMA ([84, 1024] fp16)

COMBOS = [(0, 1), (1, 0), (1, 1), (1, 2), (2, 1), (2, 2)]
OFF = {0: 0, 1: 1, 2: 4}
DIM = {0: 1, 1: 3, 2: 5}

_F32 = mybir.dt.float32
_F32R = mybir.dt.float32r
_F16 = mybir.dt.float16
_YDT = _F16
_XDT = _F16
_ODT = _F16


def _split_multi_waits(nc, max_waits=1):
    """walrus CoreV3 setupSyncWait only accepts one sync-wait per
    instruction; hoist extra waits onto same-engine NoOps placed before."""
    ctr = 0
    for fn in nc.m.functions:
        for blk in fn.blocks:
            out = []
            changed = False
            for ins in blk.instructions:
                si = getattr(ins, "sync_info", None)
                waits = list(si.on_wait) if si is not None else []
                if len(waits) > max_waits:
                    changed = True
                    keep = waits[-max_waits:]
                    for w in waits[:-max_waits]:
                        ctr += 1
                        out.append(mybir.InstNoOp(
                            name=f"I-waitsplit-{ctr}",
                            engine=ins.engine,
                            ins=[], outs=[],
                            sync_info=mybir.SyncInfo(on_wait=[w], on_update=[]),
                        ))
                    ins.sync_info = mybir.SyncInfo(
                        on_wait=keep, on_update=list(si.on_update))
                out.append(ins)
            if changed:
                blk.instructions = out
    return nc


def _build_nc():
    nc = bass.Bass()
    # host supplies slot-interleaved layout: row s*9+i = feature i of slot s
    xt_d = nc.dram_tensor("xt", [P, NBLK], _XDT, kind="ExternalInput")
    yt_d = nc.dram_tensor("yt", [P, NBLK], _YDT, kind="ExternalInput")
    cg_d = nc.dram_tensor("cgk", [P, 3 * P], _YDT, kind="ExternalInput")
    sumr_d = nc.dram_tensor("sumr", [P, TGRP * 3 * 84], _F32R, kind="ExternalInput")
    o_d = nc.dram_tensor("ov2", [OROW, N_GRP * TILE_N], _ODT, kind="ExternalOutput")

    # per-tile chunk-local column offsets and chunk boundaries
    tile_chunk = []      # (chunk_idx, local_col)
    for ci, csz in enumerate(CHUNKS):
        for j in range(csz):
            tile_chunk.append((ci, j * TILE_N))

    with tile.TileContext(nc) as tc:
        with (
            tc.tile_pool(name="consts", bufs=1) as cpool,
            tc.tile_pool(name="inx", bufs=3) as xpool,
            tc.tile_pool(name="iny", bufs=3) as ypool,
            tc.tile_pool(name="work", bufs=9) as wpool,
            tc.tile_pool(name="gstage", bufs=3) as spool,
            tc.tile_pool(name="outs", bufs=2) as opool,
            tc.tile_pool(name="psG", bufs=6, space="PSUM") as psG,
            tc.tile_pool(name="psO", bufs=2, space="PSUM") as psO,
        ):
            cg_sb = cpool.tile([P, 3 * P], _YDT, tag="cg")
            sumr_sb = cpool.tile([P, TGRP * 3 * 84], _F32R, tag="sumr")
            nc.sync.dma_start(cg_sb[:], cg_d[:])

            # PE warmup: junk matmuls reading cg (lands ~9.5us) bridge the
            # gap until real tiles stream, so the HAM clock gate promotes to
            # 2.4 GHz exactly once with no demotion dip.
            for _ in range(12):
                warm = psG.tile([P, 3 * P], _F32, tag="g")
                nc.tensor.matmul(warm[:], cg_sb[:, :P], cg_sb[:],
                                 start=True, stop=True)

            xcs, ycs = {}, {}
            t_base = 0
            for ci, csz in enumerate(CHUNKS):
                xcs[ci] = (t_base, None)
                t_base += csz

            pend = None          # deferred SUM work: (tile_idx, prods, xslice)
            out_sb = None
            opsum = None
            cur_chunk = -1

            def emit_sums(t, prods):
                nonlocal out_sb, opsum
                g, tm = t // TGRP, t % TGRP
                if tm == 0:
                    opsum = psO.tile([OROW, TILE_N], _F32, tag="o")
                    if g % OGRP == 0:
                        out_sb = opool.tile(
                            [OROW, OGRP * TILE_N], _ODT, tag="osb")
                for k in range(3):
                    v = tm * 3 + k
                    nc.tensor.matmul(
                        opsum[:], sumr_sb[:, v * 84:(v + 1) * 84],
                        prods[k][:],
                        start=(v == 0), stop=(v == TGRP * 3 - 1))
                if tm == TGRP - 1:
                    og = g % OGRP
                    nc.scalar.copy(
                        out_sb[:, og * TILE_N:(og + 1) * TILE_N], opsum[:])
                    if og == OGRP - 1:
                        p = g // OGRP
                        nc.sync.dma_start(
                            o_d[:, p * OGRP * TILE_N:(p + 1) * OGRP * TILE_N],
                            out_sb[:])

            for t in range(T):
                ci, co = tile_chunk[t]
                if ci != cur_chunk:
                    cur_chunk = ci
                    csz = CHUNKS[ci]
                    c0 = xcs[ci][0] * TILE_N
                    c1 = c0 + csz * TILE_N
                    xc = xpool.tile([P, 4 * TILE_N], _XDT, tag="xc")
                    yc = ypool.tile([P, 4 * TILE_N], _YDT, tag="yc")
                    nc.sync.dma_start(xc[:, :c1 - c0], xt_d[:, c0:c1])
                    nc.sync.dma_start(yc[:, :c1 - c0], yt_d[:, c0:c1])
                    if ci == 0:
                        # sumr is first needed by tile 0's SUM matmuls, well
                        # after the first chunk; don't delay chunk 0's issue
                        nc.sync.dma_start(sumr_sb[:], sumr_d[:])
                # G + products for tile t
                prods = []
                for k in range(3):
                    gp = psG.tile([P, TILE_N], _F32, tag="g")
                    nc.tensor.matmul(
                        gp[:], cg_sb[:, k * P:(k + 1) * P],
                        yc[:, co:co + TILE_N], start=True, stop=True)
                    prod = wpool.tile([P, TILE_N], _F32R, tag="prod")
                    if k == 2 and t < T - 2:
                        # GPSIMD can't read PSUM: ACT stages G to SBUF.
                        # Last tiles go to DVE so the tail drains fast.
                        g_sb = spool.tile([P, TILE_N], _F32, tag="gsb")
                        nc.scalar.copy(g_sb[:], gp[:])
                        nc.gpsimd.tensor_mul(
                            prod[:], g_sb[:], xc[:, co:co + TILE_N])
                    else:
                        nc.vector.tensor_mul(
                            prod[:], gp[:], xc[:, co:co + TILE_N])
                    prods.append(prod)
                # SUMs for the previous tile (1-tile software pipeline)
                if pend is not None:
                    emit_sums(*pend)
                pend = (t, prods)
            emit_sums(*pend)

    _split_multi_waits(nc)
    return nc


_NC_CACHE = None


def _get_nc():
    global _NC_CACHE
    if _NC_CACHE is None:
        _NC_CACHE = _build_nc()
    return _NC_CACHE


def _make_consts(weight, cgd):
    B = np.zeros((9, 9, 3), np.float32)
    for ci, (l1, l2) in enumerate(COMBOS):
        B[OFF[l1]:OFF[l1] + DIM[l1], OFF[l2]:OFF[l2] + DIM[l2], :] += (
            weight[ci] * cgd[(l1, l2)])
    # CGK[k][(s,j), (s,i)] = B[i,j,k]
    CGK = np.zeros((3, P, P), np.float32)
    for k in range(3):
        for s in range(S):
            CGK[k, s * 9:(s + 1) * 9, s * 9:(s + 1) * 9] = B[:, :, k].T
    cgk = np.concatenate([CGK[0], CGK[1], CGK[2]], axis=1)  # [126, 378]
    # SUMR variant (tm, k): [(s,i), tm*42 + s*3 + k] = 1
    sumr = np.zeros((P, TGRP * 3 * 84), np.float32)
    for tm in range(TGRP):
        for k in range(3):
            blk = (tm * 3 + k) * 84
            for s in range(S):
                sumr[s * 9:(s + 1) * 9, blk + tm * 42 + s * 3 + k] = 1.0
    return cgk, sumr


def _feature_major(a0, a1, a2, lo, hi):
    """[126, NBLK] slot-interleaved feature-major slice of
    concat(a0,a1,a2) rows lo:hi. Row s*9+i, col n = feature i of edge
    s*NBLK + n (relative to lo)."""
    f = np.zeros((9, E_PAD), np.float32)
    f[0, :hi - lo] = a0[lo:hi, 0]
    f[1:4, :hi - lo] = a1[lo:hi].T
    f[4:9, :hi - lo] = a2[lo:hi].T
    return np.ascontiguousarray(
        f.reshape(9, S, NBLK).transpose(1, 0, 2).reshape(P, NBLK))


def _run_spmd(inputs, trace=False):
    a0 = np.asarray(inputs["a0"], np.float32)
    a1 = np.asarray(inputs["a1"], np.float32)
    a2 = np.asarray(inputs["a2"], np.float32)
    h0 = np.asarray(inputs["h0"], np.float32)
    h1 = np.asarray(inputs["h1"], np.float32)
    h2 = np.asarray(inputs["h2"], np.float32)
    weight = np.asarray(inputs["weight"], np.float32)
    cgd = {(l1, l2): np.asarray(inputs[f"cg{l1}{l2}"], np.float32)
           for (l1, l2) in COMBOS}

    cgk, sumr = _make_consts(weight, cgd)
    ydt = mybir.dt.np(_YDT)
    xdt = mybir.dt.np(_XDT)
    cgk = cgk.astype(ydt)
    in_maps = []
    for c in range(N_CORES):
        lo, hi = c * E_CORE, (c + 1) * E_CORE
        in_maps.append({
            "xt": _feature_major(a0, a1, a2, lo, hi).astype(xdt),
            "yt": _feature_major(h0, h1, h2, lo, hi).astype(ydt),
            "cgk": cgk, "sumr": sumr,
        })

    nc = _get_nc()
    br = run_bass_kernel_spmd(nc, in_maps, list(range(N_CORES)), trace=trace)

    out = np.empty((E, 3), np.float32)
    for c in range(N_CORES):
        O = np.asarray(br.results[c]["ov2"], np.float32)  # [84, 28*512] fp16
        # row r = tm*42 + s*3 + k; col = g*512 + n; tile t = g*2 + tm
        # edge e = s*NBLK + t*512 + n
        dec = (O.reshape(TGRP, S, 3, N_GRP, TILE_N)
               .transpose(1, 3, 0, 4, 2)                # [s, g, tm, n, k]
               .reshape(E_PAD, 3))
        out[c * E_CORE:(c + 1) * E_CORE] = dec[:E_CORE]
    return out, br


def kernel(**inputs):
    out, _ = _run_spmd(inputs, trace=False)
    return out
